# revision 26
# baseline (speedup 1.0000x reference)
"""Trainium2 Bass kernel for DPS (perturbed top-k + weighted patch gather).

Per batch element b (8 total, one per NeuronCore):
  - min-max normalize scores (256 values)
  - perturbed top-k: 500 MC noise samples, top-16 of 256, sorted indices,
    one-hot mean -> soft indicators ind[k=16, d=256]
  - patches[k, c, 128, 128] = sum over 256 grid cells (i, w) of
    ind[k, i, w] * window(i, w): 128x128 windows at stride-64 offsets of
    the padded image.

Device algorithm (per core):
  Patch gather as ONE matmul over a 17x17 grid of 64x64 image tiles
  (tile index = contraction/partition dim). Each output patch quadrant
  (a, bh in {0,1}^2) contracts the grid against shifted weights
  W[k, u-a, v-bh]; all 4 quadrants x 16 k = 64 output rows share one rhs
  stream. The image arrives pre-tiled ([289, 12288] tile-major, built on
  the host while sharding) so the load is 3 large contiguous DMAs --
  strided tile gathers via DMA run at ~16 ns/descriptor (256B runs ->
  ~800 us measured) and are not viable on this hardware.

  Perturbed top-k on DVE via max8/match_replace (threshold = 16th
  largest), position-in-sorted-top16 via lower-triangular matmul (prefix
  count) on PE, indicator via per-k equality masks with fused reduction,
  weight stacking via host-provided permutation matrices on PE.
"""

import os
import numpy as np

import concourse.bacc as bacc
import concourse.tile as tile
from concourse import mybir
from concourse.bass_utils import run_bass_kernel_spmd

F32 = mybir.dt.float32
ALU = mybir.AluOpType
AX = mybir.AxisListType

N_CORES = 8
NSAMP = 500
NT = 4           # n-tiles of 125 samples
NROW = 125
D = 256
K = 16
SIGMA = 0.05

# (u, v) tile grid 17x17 = 289, flat g = 17*u + v, chunked 128/128/33
G_TILES = 289
CHUNKS = [(0, 128), (128, 256), (256, 289)]
PCOUNT = [h - l for (l, h) in CHUNKS]
NQUAD = 4  # (a, bh)
NPERM = NQUAD * 3 * 2  # quad x chunk x d-half permutation matrices
# consts layout (cols of the [128, 128*(3+NPERM)] const tensor)
C_IDENT, C_LTRI, C_LONES, C_PERM = 0, 128, 256, 384
C_BIAS = 384 + 128 * NPERM  # 17 cols: -(k+1) for k=0..15, then 1.0
CONST_W = 128 * (3 + NPERM) + 17
BLOB_W = CONST_W + NT * D + D  # consts | packed noise | scores

# main matmul free-chunking: 24 chunks of 512 cols, groups for PSUM banks
FC_GROUPS = [(0, 3), (3, 6), (6, 9), (9, 12), (12, 15), (15, 18), (18, 21), (21, 23), (23, 24)]


BF16 = mybir.dt.bfloat16
USE_BF16 = bool(int(os.environ.get("DPS_BF16", "0")))


def build_program():
    nc = bacc.Bacc(
        "TRN2", target_bir_lowering=False, debug=False, enable_asserts=False
    )

    xdt = BF16 if USE_BF16 else F32
    xt = nc.dram_tensor("xt", [G_TILES, 12288], xdt, kind="ExternalInput").ap()
    blob = nc.dram_tensor("blob", [128, BLOB_W], F32, kind="ExternalInput").ap()
    ublob = nc.dram_tensor("ublob", [128, NT * D + D], F32, kind="ExternalInput").ap()
    out = nc.dram_tensor("out", [64, 12288], F32, kind="ExternalOutput").ap()

    from contextlib import ExitStack

    with tile.TileContext(nc) as tc, ExitStack() as ctx:
        persist = ctx.enter_context(tc.tile_pool(name="persist", bufs=1))
        vwork = ctx.enter_context(tc.tile_pool(name="vwork", bufs=2))
        mwork = ctx.enter_context(tc.tile_pool(name="mwork", bufs=2))
        ework = ctx.enter_context(tc.tile_pool(name="ework", bufs=1))
        stage = ctx.enter_context(tc.tile_pool(name="stage", bufs=1))
        ps_misc = ctx.enter_context(tc.tile_pool(name="ps_misc", bufs=2, space="PSUM"))
        ps_mm = ctx.enter_context(tc.tile_pool(name="ps_mm", bufs=6, space="PSUM"))

        # ---- urgent inputs (noise+scores) first, consts second, then X ----
        ub = persist.tile([128, NT * D + D], F32, tag="ublob")
        nc.sync.dma_start(ub[:], ublob)
        bl = persist.tile([128, BLOB_W], F32, tag="blob")
        nc.sync.dma_start(bl[:, 0:CONST_W], blob[:, 0:CONST_W])
        cst = bl[:, 0:CONST_W]
        nz = ub[0:NROW, 0 : NT * D]
        s_sb = ub[0:1, NT * D : NT * D + D]

        xch = []
        for m in range(3):
            xc = persist.tile([PCOUNT[m], 12288], xdt, tag=f"xch{m}")
            xch.append(xc)
        nc.sync.dma_start(xch[0][:], xt[CHUNKS[0][0] : CHUNKS[0][1], :])
        nc.sync.dma_start(xch[1][:], xt[CHUNKS[1][0] : CHUNKS[1][1], :])
        nc.sync.dma_start(xch[2][:], xt[CHUNKS[2][0] : CHUNKS[2][1], :])

        ident = cst[:, C_IDENT : C_IDENT + 128]
        ltri = cst[:, C_LTRI : C_LTRI + 128]
        lones = cst[:, C_LONES : C_LONES + 128]

        # ---- normalize scores: snorm = (s - min) * 1/(max - min + 1e-5) ----
        smin = persist.tile([1, 1], F32, tag="smin")
        nc.vector.tensor_reduce(smin[:], s_sb, axis=AX.X, op=ALU.min)
        smax = persist.tile([1, 1], F32, tag="smax")
        nc.vector.tensor_reduce(smax[:], s_sb, axis=AX.X, op=ALU.max)
        den = persist.tile([1, 1], F32, tag="den")
        nc.vector.tensor_tensor(den[:], smax[:], smin[:], op=ALU.subtract)
        nc.vector.tensor_scalar(den[:], den[:], 1e-5, None, op0=ALU.add)
        rec = persist.tile([1, 1], F32, tag="rec")
        nc.vector.reciprocal(rec[:], den[:])
        snorm = persist.tile([1, D], F32, tag="snorm")
        nc.vector.tensor_scalar(
            snorm[:], s_sb, smin[:], rec[:], op0=ALU.subtract, op1=ALU.mult
        )

        # broadcast snorm across partitions via PE (ones @ snorm)
        ones1 = persist.tile([1, 128], F32, tag="ones1")
        nc.vector.memset(ones1[:], 1.0)
        bc_ps = ps_misc.tile([128, NSAMP], F32, tag="scr")
        nc.tensor.matmul(bc_ps[:, 0:D], ones1[:], snorm[:], start=True, stop=True)
        sbc = persist.tile([128, D], F32, tag="sbc")
        nc.vector.tensor_copy(sbc[:], bc_ps[:, 0:D])

        # ---- perturbed scores + top-16 threshold + membership, per n-tile ----
        intop = []
        for t in range(NT):
            v = vwork.tile([NROW, D], F32, tag="v")
            nc.vector.tensor_tensor(
                v[:], nz[:, t * D : (t + 1) * D], sbc[0:NROW, :], op=ALU.add
            )
            mx8a = mwork.tile([NROW, 8], F32, tag="mx8a")
            nc.vector.max(mx8a[:], v[:])
            v2 = vwork.tile([NROW, D], F32, tag="v2")
            nc.vector.match_replace(v2[:], mx8a[:], v[:], -1e30)
            mx8b = mwork.tile([NROW, 8], F32, tag="mx8b")
            nc.vector.max(mx8b[:], v2[:])
            it = persist.tile([NROW, D], F32, tag=f"intop{t}")
            nc.vector.tensor_scalar(it[:], v[:], mx8b[:, 7:8], None, op0=ALU.is_ge)
            intop.append(it)

        # ---- transpose in_top to [d, n] layout (PE) ----
        tpT = []
        for dc in range(2):
            tp_ps = ps_misc.tile([128, NSAMP], F32, tag="scr")
            for t in range(NT):
                nc.tensor.transpose(
                    tp_ps[:, t * NROW : (t + 1) * NROW],
                    intop[t][:, dc * 128 : (dc + 1) * 128],
                    ident[0:NROW, 0:NROW],
                )
            tps = persist.tile([128, NSAMP], F32, tag=f"tpT{dc}")
            nc.vector.tensor_copy(tps[:], tp_ps[:])
            tpT.append(tps)

        # ---- pos[d, n] = # of in-top cells with smaller index ----
        # Z = in_top * (pos + 1) in {0, 1..16}; ind[k] = mean_n(Z == k+1)
        Z = []
        pp0 = ps_misc.tile([128, NSAMP], F32, tag="scr")
        nc.tensor.matmul(pp0[:], ltri, tpT[0][:], start=True, stop=True)
        pp1 = ps_misc.tile([128, NSAMP], F32, tag="scr")
        nc.tensor.matmul(pp1[:], lones, tpT[0][:], start=True, stop=False)
        nc.tensor.matmul(pp1[:], ltri, tpT[1][:], start=False, stop=True)
        for dc, pp in ((0, pp0), (1, pp1)):
            z0 = ework.tile([128, NSAMP], F32, tag="z0")
            nc.vector.tensor_tensor(z0[:], pp[:], tpT[dc][:], op=ALU.mult)
            zt = persist.tile([128, NSAMP], BF16, tag=f"z{dc}")
            nc.vector.tensor_tensor(zt[:], z0[:], tpT[dc][:], op=ALU.add)
            Z.append(zt)

        # ---- indicators: ind_sb[p, 16*dc + k] = ind[k, d = 128*dc + p] ----
        ind_sb = persist.tile([128, 32], F32, tag="ind")
        use_accum = bool(int(os.environ.get("DPS_TS_ACCUM", "1")))
        ACT_MASKS = {(1, k) for k in range(6, 16)}
        for dc in range(2):
            for k in range(K):
                col = 16 * dc + k
                if (dc, k) in ACT_MASKS:
                    h1 = ework.tile([128, NSAMP], BF16, tag="h1")
                    nc.scalar.activation(
                        h1[:], Z[dc][:], mybir.ActivationFunctionType.Abs,
                        bias=cst[:, C_BIAS + k : C_BIAS + k + 1], scale=1.0,
                    )
                    h2 = ework.tile([128, NSAMP], BF16, tag="h2")
                    nc.scalar.activation(
                        h2[:], h1[:], mybir.ActivationFunctionType.Relu,
                        bias=cst[:, C_BIAS + 16 : C_BIAS + 17], scale=-1.0,
                        accum_out=ind_sb[:, col : col + 1],
                    )
                elif use_accum:
                    junk = ework.tile([128, NSAMP], F32, tag="junk")
                    nc.vector.tensor_scalar(
                        junk[:],
                        Z[dc][:],
                        float(k + 1),
                        None,
                        op0=ALU.is_equal,
                        op1=ALU.add,
                        accum_out=ind_sb[:, col : col + 1],
                    )
                else:
                    eq = ework.tile([128, NSAMP], F32, tag="junk")
                    nc.vector.tensor_scalar(
                        eq[:], Z[dc][:], float(k + 1), None, op0=ALU.is_equal
                    )
                    nc.vector.tensor_reduce(
                        ind_sb[:, col : col + 1], eq[:], axis=AX.X, op=ALU.add
                    )
        nc.scalar.mul(ind_sb[:], ind_sb[:], 1.0 / NSAMP)

        # ---- Wstack via permutation matmuls ----
        # wst[m][g - 128*m, 16*quad + k] = ind[k, 16*(u-a) + (v-bh)],
        #   g = 17*u + v, quad = 2*a + bh
        wst = []
        for m in range(3):
            w_ps = ps_misc.tile([128, NSAMP], F32, tag="scr")
            for quad in range(NQUAD):
                for dc in range(2):
                    j = (quad * 3 + m) * 2 + dc
                    perm = cst[:, C_PERM + 128 * j : C_PERM + 128 * j + PCOUNT[m]]
                    nc.tensor.matmul(
                        w_ps[0 : PCOUNT[m], 16 * quad : 16 * quad + 16],
                        perm,
                        ind_sb[:, 16 * dc : 16 * dc + 16],
                        start=(dc == 0),
                        stop=(dc == 1),
                    )
            w = persist.tile([PCOUNT[m], 64], xdt, tag=f"wst{m}")
            nc.scalar.copy(w[:], w_ps[0 : PCOUNT[m], 0:64])
            wst.append(w)

        # ---- main matmul: out[64, 12288] = Wstack.T @ Xtiles ----
        for g0, g1 in FC_GROUPS:
            ptiles = []
            for fc in range(g0, g1):
                pt = ps_mm.tile([64, 512], F32, tag="mm")
                ptiles.append((fc, pt))
            for cc in range(3):
                for fc, pt in ptiles:
                    nc.tensor.matmul(
                        pt[:],
                        wst[cc][:],
                        xch[cc][:, fc * 512 : (fc + 1) * 512],
                        start=(cc == 0),
                        stop=(cc == 2),
                    )
            st = stage.tile([64, 512 * (g1 - g0)], F32, tag="st")
            for fc, pt in ptiles:
                nc.scalar.copy(st[:, (fc - g0) * 512 : (fc - g0 + 1) * 512], pt[:])
            nc.sync.dma_start(out[:, g0 * 512 : g1 * 512], st[:])

    nc.compile()
    return nc


_NC = None


def _get_program():
    global _NC
    if _NC is None:
        _NC = build_program()
    return _NC


def _host_consts():
    import jax

    cpu = jax.devices("cpu")[0]
    with jax.default_device(cpu):
        key = jax.random.key(42)
        noise = jax.random.normal(key, (N_CORES, NSAMP, D), dtype=np.float32)
        noise = np.asarray(noise) * np.float32(SIGMA)

    blocks = [
        np.eye(128, dtype=np.float32),
        np.triu(np.ones((128, 128), np.float32), k=1),  # [d', d]: 1 iff d' < d
        np.ones((128, 128), np.float32),
    ]
    for quad in range(NQUAD):
        a, bh = quad >> 1, quad & 1
        for m in range(3):
            for dc in range(2):
                P = np.zeros((128, 128), np.float32)
                for i in range(16):
                    for w in range(16):
                        d = 16 * i + w
                        if d // 128 != dc:
                            continue
                        g = 17 * (i + a) + (w + bh)
                        gl = g - 128 * m
                        if 0 <= gl < PCOUNT[m]:
                            P[d % 128, gl] = 1.0
                blocks.append(P)
    bias = np.zeros((128, 17), np.float32)
    for k in range(16):
        bias[:, k] = -(k + 1.0)
    bias[:, 16] = 1.0
    blocks.append(bias)
    consts = np.concatenate(blocks, axis=1)
    return noise, np.ascontiguousarray(consts)


def _prep_core(x_core, scores_core, noise_core, consts):
    """x_core [3,1024,1024] -> tile-major [289, 12288]; pack noise."""
    xp = np.zeros((3, 1088, 1088), np.float32)
    xp[:, 32:1056, 32:1056] = x_core
    xtm = (
        xp.reshape(3, 17, 64, 17, 64)
        .transpose(1, 3, 0, 2, 4)
        .reshape(G_TILES, 12288)
    )
    nzp = noise_core.reshape(NT, NROW, D).transpose(1, 0, 2).reshape(NROW, NT * D)
    blob = np.zeros((128, BLOB_W), np.float32)
    blob[:, 0:CONST_W] = consts
    ublob = np.zeros((128, NT * D + D), np.float32)
    ublob[0:NROW, 0 : NT * D] = nzp
    ublob[0, NT * D : NT * D + D] = scores_core.reshape(D)
    if USE_BF16:
        import ml_dtypes

        xtm = xtm.astype(ml_dtypes.bfloat16)
    return {"xt": np.ascontiguousarray(xtm), "blob": blob, "ublob": ublob}


def kernel(x_high: np.ndarray, scores_2d: np.ndarray) -> np.ndarray:
    nc = _get_program()
    noise, consts = _host_consts()
    x_high = np.asarray(x_high, dtype=np.float32)
    scores_2d = np.asarray(scores_2d, dtype=np.float32)
    in_maps = [
        _prep_core(x_high[c], scores_2d[c], noise[c], consts)
        for c in range(N_CORES)
    ]
    res = run_bass_kernel_spmd(nc, in_maps, list(range(N_CORES)))
    outs = []
    for c in range(N_CORES):
        o = res.results[c]["out"]  # [64, 12288] rows = 16*quad + k
        o = o.reshape(2, 2, 16, 3, 64, 64).transpose(2, 3, 0, 4, 1, 5)
        outs.append(o.reshape(16, 3, 128, 128))
    return np.stack(outs).reshape(128, 3, 128, 128)


# revision 28
# speedup vs baseline: 1.0106x; 1.0106x over previous
"""Trainium2 Bass kernel for DPS (perturbed top-k + weighted patch gather).

Per batch element b (8 total, one per NeuronCore):
  - min-max normalize scores (256 values)
  - perturbed top-k: 500 MC noise samples, top-16 of 256, sorted indices,
    one-hot mean -> soft indicators ind[k=16, d=256]
  - patches[k, c, 128, 128] = sum over 256 grid cells (i, w) of
    ind[k, i, w] * window(i, w): 128x128 windows at stride-64 offsets of
    the padded image.

Device algorithm (per core):
  Patch gather as ONE matmul over a 17x17 grid of 64x64 image tiles
  (tile index = contraction/partition dim). Each output patch quadrant
  (a, bh in {0,1}^2) contracts the grid against shifted weights
  W[k, u-a, v-bh]; all 4 quadrants x 16 k = 64 output rows share one rhs
  stream. The image arrives pre-tiled ([289, 12288] tile-major, built on
  the host while sharding) so the load is 3 large contiguous DMAs --
  strided tile gathers via DMA run at ~16 ns/descriptor (256B runs ->
  ~800 us measured) and are not viable on this hardware.

  Perturbed top-k on DVE via max8/match_replace (threshold = 16th
  largest), position-in-sorted-top16 via lower-triangular matmul (prefix
  count) on PE, indicator via per-k equality masks with fused reduction,
  weight stacking via host-provided permutation matrices on PE.
"""

import os
import numpy as np

import concourse.bacc as bacc
import concourse.tile as tile
from concourse import mybir
from concourse.bass_utils import run_bass_kernel_spmd

F32 = mybir.dt.float32
ALU = mybir.AluOpType
AX = mybir.AxisListType

N_CORES = 8
NSAMP = 500
NT = 4           # n-tiles of 125 samples
NROW = 125
D = 256
K = 16
SIGMA = 0.05

# (u, v) tile grid 17x17 = 289, flat g = 17*u + v, chunked 128/128/33
G_TILES = 289
CHUNKS = [(0, 128), (128, 256), (256, 289)]
PCOUNT = [h - l for (l, h) in CHUNKS]
NQUAD = 4  # (a, bh)
NPERM = NQUAD * 3 * 2  # quad x chunk x d-half permutation matrices
# consts layout (cols of the [128, 128*(3+NPERM)] const tensor)
C_IDENT, C_LTRI, C_LONES, C_PERM = 0, 128, 256, 384
C_BIAS = 384 + 128 * NPERM  # 17 cols: -(k+1) for k=0..15, then 1.0
CONST_W = 128 * (3 + NPERM) + 17
BLOB_W = CONST_W + NT * D + D  # consts | packed noise | scores

# main matmul free-chunking: 24 chunks of 512 cols, groups for PSUM banks
FCW = 512
FC_GROUPS = [(0, 3), (3, 6), (6, 9), (9, 12), (12, 15), (15, 18), (18, 21), (21, 23), (23, 24)]


BF16 = mybir.dt.bfloat16
USE_BF16 = bool(int(os.environ.get("DPS_BF16", "0")))


def build_program():
    nc = bacc.Bacc(
        "TRN2", target_bir_lowering=False, debug=False, enable_asserts=False
    )

    xdt = BF16 if USE_BF16 else F32
    xt = nc.dram_tensor("xt", [G_TILES, 12288], xdt, kind="ExternalInput").ap()
    blob = nc.dram_tensor("blob", [128, BLOB_W], F32, kind="ExternalInput").ap()
    ublob = nc.dram_tensor("ublob", [128, NT * D + D], F32, kind="ExternalInput").ap()
    out = nc.dram_tensor("out", [64, 12288], F32, kind="ExternalOutput").ap()

    from contextlib import ExitStack

    with tile.TileContext(nc) as tc, ExitStack() as ctx:
        persist = ctx.enter_context(tc.tile_pool(name="persist", bufs=1))
        vwork = ctx.enter_context(tc.tile_pool(name="vwork", bufs=2))
        mwork = ctx.enter_context(tc.tile_pool(name="mwork", bufs=2))
        ework = ctx.enter_context(tc.tile_pool(name="ework", bufs=1))
        stage = ctx.enter_context(tc.tile_pool(name="stage", bufs=1))
        ps_misc = ctx.enter_context(tc.tile_pool(name="ps_misc", bufs=2, space="PSUM"))
        ps_mm = ctx.enter_context(tc.tile_pool(name="ps_mm", bufs=6, space="PSUM"))

        # ---- urgent inputs (noise+scores) first, consts second, then X ----
        ub = persist.tile([128, NT * D + D], F32, tag="ublob")
        nc.sync.dma_start(ub[:], ublob)
        bl = persist.tile([128, BLOB_W], F32, tag="blob")
        nc.sync.dma_start(bl[:, 0:CONST_W], blob[:, 0:CONST_W])
        cst = bl[:, 0:CONST_W]
        nz = ub[0:NROW, 0 : NT * D]
        s_sb = ub[0:1, NT * D : NT * D + D]

        xch = []
        for m in range(3):
            xc = persist.tile([PCOUNT[m], 12288], xdt, tag=f"xch{m}")
            xch.append(xc)
        nc.sync.dma_start(xch[0][:], xt[CHUNKS[0][0] : CHUNKS[0][1], :])
        nc.sync.dma_start(xch[1][:], xt[CHUNKS[1][0] : CHUNKS[1][1], :])
        nc.sync.dma_start(xch[2][:], xt[CHUNKS[2][0] : CHUNKS[2][1], :])

        ident = cst[:, C_IDENT : C_IDENT + 128]
        ltri = cst[:, C_LTRI : C_LTRI + 128]
        lones = cst[:, C_LONES : C_LONES + 128]

        # ---- normalize scores: snorm = (s - min) * 1/(max - min + 1e-5) ----
        smin = persist.tile([1, 1], F32, tag="smin")
        nc.vector.tensor_reduce(smin[:], s_sb, axis=AX.X, op=ALU.min)
        smax = persist.tile([1, 1], F32, tag="smax")
        nc.vector.tensor_reduce(smax[:], s_sb, axis=AX.X, op=ALU.max)
        den = persist.tile([1, 1], F32, tag="den")
        nc.vector.tensor_tensor(den[:], smax[:], smin[:], op=ALU.subtract)
        nc.vector.tensor_scalar(den[:], den[:], 1e-5, None, op0=ALU.add)
        rec = persist.tile([1, 1], F32, tag="rec")
        nc.vector.reciprocal(rec[:], den[:])
        snorm = persist.tile([1, D], F32, tag="snorm")
        nc.vector.tensor_scalar(
            snorm[:], s_sb, smin[:], rec[:], op0=ALU.subtract, op1=ALU.mult
        )

        # broadcast snorm across partitions via PE (ones @ snorm)
        ones1 = persist.tile([1, 128], F32, tag="ones1")
        nc.vector.memset(ones1[:], 1.0)
        bc_ps = ps_misc.tile([128, NSAMP], F32, tag="scr")
        nc.tensor.matmul(bc_ps[:, 0:D], ones1[:], snorm[:], start=True, stop=True)
        sbc = persist.tile([128, D], F32, tag="sbc")
        nc.vector.tensor_copy(sbc[:], bc_ps[:, 0:D])

        # ---- perturbed scores + top-16 threshold + membership, per n-tile ----
        intop = []
        for t in range(NT):
            v = vwork.tile([NROW, D], F32, tag="v")
            nc.vector.tensor_tensor(
                v[:], nz[:, t * D : (t + 1) * D], sbc[0:NROW, :], op=ALU.add
            )
            mx8a = mwork.tile([NROW, 8], F32, tag="mx8a")
            nc.vector.max(mx8a[:], v[:])
            v2 = vwork.tile([NROW, D], F32, tag="v2")
            nc.vector.match_replace(v2[:], mx8a[:], v[:], -1e30)
            mx8b = mwork.tile([NROW, 8], F32, tag="mx8b")
            nc.vector.max(mx8b[:], v2[:])
            it = persist.tile([NROW, D], F32, tag=f"intop{t}")
            nc.vector.tensor_scalar(it[:], v[:], mx8b[:, 7:8], None, op0=ALU.is_ge)
            intop.append(it)

        # ---- transpose in_top to [d, n] layout (PE) ----
        tpT = []
        for dc in range(2):
            tp_ps = ps_misc.tile([128, NSAMP], F32, tag="scr")
            for t in range(NT):
                nc.tensor.transpose(
                    tp_ps[:, t * NROW : (t + 1) * NROW],
                    intop[t][:, dc * 128 : (dc + 1) * 128],
                    ident[0:NROW, 0:NROW],
                )
            tps = persist.tile([128, NSAMP], F32, tag=f"tpT{dc}")
            nc.vector.tensor_copy(tps[:], tp_ps[:])
            tpT.append(tps)

        # ---- pos[d, n] = # of in-top cells with smaller index ----
        # Z = in_top * (pos + 1) in {0, 1..16}; ind[k] = mean_n(Z == k+1)
        Z = []
        pp0 = ps_misc.tile([128, NSAMP], F32, tag="scr")
        nc.tensor.matmul(pp0[:], ltri, tpT[0][:], start=True, stop=True)
        pp1 = ps_misc.tile([128, NSAMP], F32, tag="scr")
        nc.tensor.matmul(pp1[:], lones, tpT[0][:], start=True, stop=False)
        nc.tensor.matmul(pp1[:], ltri, tpT[1][:], start=False, stop=True)
        for dc, pp in ((0, pp0), (1, pp1)):
            z0 = ework.tile([128, NSAMP], F32, tag="z0")
            nc.vector.tensor_tensor(z0[:], pp[:], tpT[dc][:], op=ALU.mult)
            zt = persist.tile([128, NSAMP], BF16, tag=f"z{dc}")
            nc.vector.tensor_tensor(zt[:], z0[:], tpT[dc][:], op=ALU.add)
            Z.append(zt)

        # ---- indicators: ind_sb[p, 16*dc + k] = ind[k, d = 128*dc + p] ----
        ind_sb = persist.tile([128, 32], F32, tag="ind")
        use_accum = bool(int(os.environ.get("DPS_TS_ACCUM", "1")))
        ACT_MASKS = {(1, k) for k in range(6, 16)}
        for dc in range(2):
            for k in range(K):
                col = 16 * dc + k
                if (dc, k) in ACT_MASKS:
                    h1 = ework.tile([128, NSAMP], BF16, tag="h1")
                    nc.scalar.activation(
                        h1[:], Z[dc][:], mybir.ActivationFunctionType.Abs,
                        bias=cst[:, C_BIAS + k : C_BIAS + k + 1], scale=1.0,
                    )
                    h2 = ework.tile([128, NSAMP], BF16, tag="h2")
                    nc.scalar.activation(
                        h2[:], h1[:], mybir.ActivationFunctionType.Relu,
                        bias=cst[:, C_BIAS + 16 : C_BIAS + 17], scale=-1.0,
                        accum_out=ind_sb[:, col : col + 1],
                    )
                elif use_accum:
                    junk = ework.tile([128, NSAMP], F32, tag="junk")
                    nc.vector.tensor_scalar(
                        junk[:],
                        Z[dc][:],
                        float(k + 1),
                        None,
                        op0=ALU.is_equal,
                        op1=ALU.add,
                        accum_out=ind_sb[:, col : col + 1],
                    )
                else:
                    eq = ework.tile([128, NSAMP], F32, tag="junk")
                    nc.vector.tensor_scalar(
                        eq[:], Z[dc][:], float(k + 1), None, op0=ALU.is_equal
                    )
                    nc.vector.tensor_reduce(
                        ind_sb[:, col : col + 1], eq[:], axis=AX.X, op=ALU.add
                    )
        nc.scalar.mul(ind_sb[:], ind_sb[:], 1.0 / NSAMP)

        # ---- Wstack via permutation matmuls ----
        # wst[m][g - 128*m, 16*quad + k] = ind[k, 16*(u-a) + (v-bh)],
        #   g = 17*u + v, quad = 2*a + bh
        wst = []
        for m in range(3):
            w_ps = ps_misc.tile([128, NSAMP], F32, tag="scr")
            for quad in range(NQUAD):
                for dc in range(2):
                    j = (quad * 3 + m) * 2 + dc
                    perm = cst[:, C_PERM + 128 * j : C_PERM + 128 * j + PCOUNT[m]]
                    nc.tensor.matmul(
                        w_ps[0 : PCOUNT[m], 16 * quad : 16 * quad + 16],
                        perm,
                        ind_sb[:, 16 * dc : 16 * dc + 16],
                        start=(dc == 0),
                        stop=(dc == 1),
                    )
            w = persist.tile([PCOUNT[m], 64], xdt, tag=f"wst{m}")
            nc.scalar.copy(w[:], w_ps[0 : PCOUNT[m], 0:64])
            wst.append(w)

        # ---- main matmul: out[64, 12288] = Wstack.T @ Xtiles ----
        for g0, g1 in FC_GROUPS:
            ptiles = []
            for fc in range(g0, g1):
                pt = ps_mm.tile([64, FCW], F32, tag="mm")
                ptiles.append((fc, pt))
            for cc in range(3):
                for fc, pt in ptiles:
                    nc.tensor.matmul(
                        pt[:],
                        wst[cc][:],
                        xch[cc][:, fc * FCW : (fc + 1) * FCW],
                        start=(cc == 0),
                        stop=(cc == 2),
                    )
            st = stage.tile([64, FCW * (g1 - g0)], F32, tag="st")
            for fc, pt in ptiles:
                nc.scalar.copy(st[:, (fc - g0) * FCW : (fc - g0 + 1) * FCW], pt[:])
            nc.sync.dma_start(out[:, g0 * FCW : g1 * FCW], st[:])

    nc.compile()
    return nc


_NC = None


def _get_program():
    global _NC
    if _NC is None:
        _NC = build_program()
    return _NC


def _host_consts():
    import jax

    cpu = jax.devices("cpu")[0]
    with jax.default_device(cpu):
        key = jax.random.key(42)
        noise = jax.random.normal(key, (N_CORES, NSAMP, D), dtype=np.float32)
        noise = np.asarray(noise) * np.float32(SIGMA)

    blocks = [
        np.eye(128, dtype=np.float32),
        np.triu(np.ones((128, 128), np.float32), k=1),  # [d', d]: 1 iff d' < d
        np.ones((128, 128), np.float32),
    ]
    for quad in range(NQUAD):
        a, bh = quad >> 1, quad & 1
        for m in range(3):
            for dc in range(2):
                P = np.zeros((128, 128), np.float32)
                for i in range(16):
                    for w in range(16):
                        d = 16 * i + w
                        if d // 128 != dc:
                            continue
                        g = 17 * (i + a) + (w + bh)
                        gl = g - 128 * m
                        if 0 <= gl < PCOUNT[m]:
                            P[d % 128, gl] = 1.0
                blocks.append(P)
    bias = np.zeros((128, 17), np.float32)
    for k in range(16):
        bias[:, k] = -(k + 1.0)
    bias[:, 16] = 1.0
    blocks.append(bias)
    consts = np.concatenate(blocks, axis=1)
    return noise, np.ascontiguousarray(consts)


def _prep_core(x_core, scores_core, noise_core, consts):
    """x_core [3,1024,1024] -> tile-major [289, 12288]; pack noise."""
    xp = np.zeros((3, 1088, 1088), np.float32)
    xp[:, 32:1056, 32:1056] = x_core
    xtm = (
        xp.reshape(3, 17, 64, 17, 64)
        .transpose(1, 3, 0, 2, 4)
        .reshape(G_TILES, 12288)
    )
    nzp = noise_core.reshape(NT, NROW, D).transpose(1, 0, 2).reshape(NROW, NT * D)
    blob = np.zeros((128, BLOB_W), np.float32)
    blob[:, 0:CONST_W] = consts
    ublob = np.zeros((128, NT * D + D), np.float32)
    ublob[0:NROW, 0 : NT * D] = nzp
    ublob[0, NT * D : NT * D + D] = scores_core.reshape(D)
    if USE_BF16:
        import ml_dtypes

        xtm = xtm.astype(ml_dtypes.bfloat16)
    return {"xt": np.ascontiguousarray(xtm), "blob": blob, "ublob": ublob}


def kernel(x_high: np.ndarray, scores_2d: np.ndarray) -> np.ndarray:
    nc = _get_program()
    noise, consts = _host_consts()
    x_high = np.asarray(x_high, dtype=np.float32)
    scores_2d = np.asarray(scores_2d, dtype=np.float32)
    in_maps = [
        _prep_core(x_high[c], scores_2d[c], noise[c], consts)
        for c in range(N_CORES)
    ]
    res = run_bass_kernel_spmd(nc, in_maps, list(range(N_CORES)))
    outs = []
    for c in range(N_CORES):
        o = res.results[c]["out"]  # [64, 12288] rows = 16*quad + k
        o = o.reshape(2, 2, 16, 3, 64, 64).transpose(2, 3, 0, 4, 1, 5)
        outs.append(o.reshape(16, 3, 128, 128))
    return np.stack(outs).reshape(128, 3, 128, 128)


# revision 30
# speedup vs baseline: 1.0107x; 1.0001x over previous
"""Trainium2 Bass kernel for DPS (perturbed top-k + weighted patch gather).

Per batch element b (8 total, one per NeuronCore):
  - min-max normalize scores (256 values)
  - perturbed top-k: 500 MC noise samples, top-16 of 256, sorted indices,
    one-hot mean -> soft indicators ind[k=16, d=256]
  - patches[k, c, 128, 128] = sum over 256 grid cells (i, w) of
    ind[k, i, w] * window(i, w): 128x128 windows at stride-64 offsets of
    the padded image.

Device algorithm (per core):
  Patch gather as ONE matmul over a 17x17 grid of 64x64 image tiles
  (tile index = contraction/partition dim). Each output patch quadrant
  (a, bh in {0,1}^2) contracts the grid against shifted weights
  W[k, u-a, v-bh]; all 4 quadrants x 16 k = 64 output rows share one rhs
  stream. The image arrives pre-tiled ([289, 12288] tile-major, built on
  the host while sharding) so the load is 3 large contiguous DMAs --
  strided tile gathers via DMA run at ~16 ns/descriptor (256B runs ->
  ~800 us measured) and are not viable on this hardware.

  Perturbed top-k on DVE via max8/match_replace (threshold = 16th
  largest), position-in-sorted-top16 via lower-triangular matmul (prefix
  count) on PE, indicator via per-k equality masks with fused reduction,
  weight stacking via host-provided permutation matrices on PE.
"""

import os
import numpy as np

import concourse.bacc as bacc
import concourse.tile as tile
from concourse import mybir
from concourse.bass_utils import run_bass_kernel_spmd

F32 = mybir.dt.float32
ALU = mybir.AluOpType
AX = mybir.AxisListType

N_CORES = 8
NSAMP = 500
NT = 4           # n-tiles of 125 samples
NROW = 125
D = 256
K = 16
SIGMA = 0.05

# (u, v) tile grid 17x17 = 289, flat g = 17*u + v, chunked 128/128/33
G_TILES = 289
CHUNKS = [(0, 128), (128, 256), (256, 289)]
PCOUNT = [h - l for (l, h) in CHUNKS]
NQUAD = 4  # (a, bh)
NPERM = NQUAD * 3 * 2  # quad x chunk x d-half permutation matrices
# consts layout (cols of the [128, 128*(3+NPERM)] const tensor)
C_IDENT, C_LTRI, C_LONES, C_PERM = 0, 128, 256, 384
C_BIAS = 384 + 128 * NPERM  # 17 cols: -(k+1) for k=0..15, then 1.0
CONST_W = 128 * (3 + NPERM) + 17
BLOB_W = CONST_W + NT * D + D  # consts | packed noise | scores

# main matmul free-chunking: 24 chunks of 512 cols, groups for PSUM banks
FCW = 512
FC_GROUPS = [(0, 3), (3, 6), (6, 9), (9, 12), (12, 15), (15, 18), (18, 21), (21, 23), (23, 24)]


BF16 = mybir.dt.bfloat16
USE_BF16 = bool(int(os.environ.get("DPS_BF16", "0")))


def build_program():
    nc = bacc.Bacc(
        "TRN2", target_bir_lowering=False, debug=False, enable_asserts=False
    )

    xdt = BF16 if USE_BF16 else F32
    xt = nc.dram_tensor("xt", [G_TILES, 12288], xdt, kind="ExternalInput").ap()
    blob = nc.dram_tensor("blob", [128, BLOB_W], F32, kind="ExternalInput").ap()
    ublob = nc.dram_tensor("ublob", [128, NT * D + D], F32, kind="ExternalInput").ap()
    out = nc.dram_tensor("out", [64, 12288], F32, kind="ExternalOutput").ap()

    from contextlib import ExitStack

    with tile.TileContext(nc) as tc, ExitStack() as ctx:
        persist = ctx.enter_context(tc.tile_pool(name="persist", bufs=1))
        vwork = ctx.enter_context(tc.tile_pool(name="vwork", bufs=2))
        mwork = ctx.enter_context(tc.tile_pool(name="mwork", bufs=2))
        ework = ctx.enter_context(tc.tile_pool(name="ework", bufs=1))
        stage = ctx.enter_context(tc.tile_pool(name="stage", bufs=1))
        ps_misc = ctx.enter_context(tc.tile_pool(name="ps_misc", bufs=2, space="PSUM"))
        ps_mm = ctx.enter_context(tc.tile_pool(name="ps_mm", bufs=6, space="PSUM"))

        # ---- urgent inputs (noise+scores) first, consts second, then X ----
        ub = persist.tile([128, NT * D + D], F32, tag="ublob")
        nc.sync.dma_start(ub[:], ublob)
        bl = persist.tile([128, BLOB_W], F32, tag="blob")
        nc.sync.dma_start(bl[:, 0:CONST_W], blob[:, 0:CONST_W])
        cst = bl[:, 0:CONST_W]
        nz = ub[0:NROW, 0 : NT * D]
        s_sb = ub[0:1, NT * D : NT * D + D]

        xch = []
        for m in range(3):
            xc = persist.tile([PCOUNT[m], 12288], xdt, tag=f"xch{m}")
            xch.append(xc)
        nc.sync.dma_start(xch[0][:], xt[CHUNKS[0][0] : CHUNKS[0][1], :])
        nc.sync.dma_start(xch[1][:], xt[CHUNKS[1][0] : CHUNKS[1][1], :])
        nc.sync.dma_start(xch[2][:], xt[CHUNKS[2][0] : CHUNKS[2][1], :])

        ident = cst[:, C_IDENT : C_IDENT + 128]
        ltri = cst[:, C_LTRI : C_LTRI + 128]
        lones = cst[:, C_LONES : C_LONES + 128]

        # ---- normalize scores: snorm = (s - min) * 1/(max - min + 1e-5) ----
        smin = persist.tile([1, 1], F32, tag="smin")
        nc.vector.tensor_reduce(smin[:], s_sb, axis=AX.X, op=ALU.min)
        smax = persist.tile([1, 1], F32, tag="smax")
        nc.vector.tensor_reduce(smax[:], s_sb, axis=AX.X, op=ALU.max)
        den = persist.tile([1, 1], F32, tag="den")
        nc.vector.tensor_tensor(den[:], smax[:], smin[:], op=ALU.subtract)
        nc.vector.tensor_scalar(den[:], den[:], 1e-5, None, op0=ALU.add)
        rec = persist.tile([1, 1], F32, tag="rec")
        nc.vector.reciprocal(rec[:], den[:])
        snorm = persist.tile([1, D], F32, tag="snorm")
        nc.vector.tensor_scalar(
            snorm[:], s_sb, smin[:], rec[:], op0=ALU.subtract, op1=ALU.mult
        )

        # broadcast snorm across partitions via PE (ones @ snorm)
        ones1 = persist.tile([1, 128], F32, tag="ones1")
        nc.vector.memset(ones1[:], 1.0)
        bc_ps = ps_misc.tile([128, NSAMP], F32, tag="scr")
        nc.tensor.matmul(bc_ps[:, 0:D], ones1[:], snorm[:], start=True, stop=True)
        sbc = persist.tile([128, D], F32, tag="sbc")
        nc.vector.tensor_copy(sbc[:], bc_ps[:, 0:D])

        # ---- perturbed scores + top-16 threshold + membership, per n-tile ----
        intop = []
        for t in range(NT):
            v = vwork.tile([NROW, D], F32, tag="v")
            nc.vector.tensor_tensor(
                v[:], nz[:, t * D : (t + 1) * D], sbc[0:NROW, :], op=ALU.add
            )
            mx8a = mwork.tile([NROW, 8], F32, tag="mx8a")
            nc.vector.max(mx8a[:], v[:])
            v2 = vwork.tile([NROW, D], F32, tag="v2")
            nc.vector.match_replace(v2[:], mx8a[:], v[:], -1e30)
            mx8b = mwork.tile([NROW, 8], F32, tag="mx8b")
            nc.vector.max(mx8b[:], v2[:])
            it = persist.tile([NROW, D], F32, tag=f"intop{t}")
            nc.vector.tensor_scalar(it[:], v[:], mx8b[:, 7:8], None, op0=ALU.is_ge)
            intop.append(it)

        # ---- transpose in_top to [d, n] layout (PE) ----
        tpT = []
        for dc in range(2):
            tp_ps = ps_misc.tile([128, NSAMP], F32, tag="scr")
            for t in range(NT):
                nc.tensor.transpose(
                    tp_ps[:, t * NROW : (t + 1) * NROW],
                    intop[t][:, dc * 128 : (dc + 1) * 128],
                    ident[0:NROW, 0:NROW],
                )
            tps = persist.tile([128, NSAMP], F32, tag=f"tpT{dc}")
            nc.vector.tensor_copy(tps[:], tp_ps[:])
            tpT.append(tps)

        # ---- pos[d, n] = # of in-top cells with smaller index ----
        # Z = in_top * (pos + 1) in {0, 1..16}; ind[k] = mean_n(Z == k+1)
        Z = []
        pp0 = ps_misc.tile([128, NSAMP], F32, tag="scr")
        nc.tensor.matmul(pp0[:], ltri, tpT[0][:], start=True, stop=True)
        pp1 = ps_misc.tile([128, NSAMP], F32, tag="scr")
        nc.tensor.matmul(pp1[:], lones, tpT[0][:], start=True, stop=False)
        nc.tensor.matmul(pp1[:], ltri, tpT[1][:], start=False, stop=True)
        for dc, pp in ((0, pp0), (1, pp1)):
            z0 = ework.tile([128, NSAMP], F32, tag="z0")
            nc.vector.tensor_tensor(z0[:], pp[:], tpT[dc][:], op=ALU.mult)
            zt = persist.tile([128, NSAMP], BF16, tag=f"z{dc}")
            nc.vector.tensor_tensor(zt[:], z0[:], tpT[dc][:], op=ALU.add)
            Z.append(zt)

        # ---- indicators: ind_sb[p, 16*dc + k] = ind[k, d = 128*dc + p] ----
        ind_sb = persist.tile([128, 32], F32, tag="ind")
        use_accum = bool(int(os.environ.get("DPS_TS_ACCUM", "1")))
        ACT_MASKS = {(1, k) for k in range(6, 16)}
        for dc in range(2):
            for k in range(K):
                col = 16 * dc + k
                if (dc, k) in ACT_MASKS:
                    h1 = ework.tile([128, NSAMP], BF16, tag="h1")
                    nc.scalar.activation(
                        h1[:], Z[dc][:], mybir.ActivationFunctionType.Abs,
                        bias=cst[:, C_BIAS + k : C_BIAS + k + 1], scale=1.0,
                    )
                    h2 = ework.tile([128, NSAMP], BF16, tag="h2")
                    nc.scalar.activation(
                        h2[:], h1[:], mybir.ActivationFunctionType.Relu,
                        bias=cst[:, C_BIAS + 16 : C_BIAS + 17], scale=-1.0,
                        accum_out=ind_sb[:, col : col + 1],
                    )
                elif use_accum:
                    junk = ework.tile([128, NSAMP], F32, tag="junk")
                    nc.vector.tensor_scalar(
                        junk[:],
                        Z[dc][:],
                        float(k + 1),
                        None,
                        op0=ALU.is_equal,
                        op1=ALU.add,
                        accum_out=ind_sb[:, col : col + 1],
                    )
                else:
                    eq = ework.tile([128, NSAMP], F32, tag="junk")
                    nc.vector.tensor_scalar(
                        eq[:], Z[dc][:], float(k + 1), None, op0=ALU.is_equal
                    )
                    nc.vector.tensor_reduce(
                        ind_sb[:, col : col + 1], eq[:], axis=AX.X, op=ALU.add
                    )
        nc.scalar.mul(ind_sb[:], ind_sb[:], 1.0 / NSAMP)

        # ---- Wstack via permutation matmuls ----
        # wst[m][g - 128*m, 16*quad + k] = ind[k, 16*(u-a) + (v-bh)],
        #   g = 17*u + v, quad = 2*a + bh
        wst = []
        for m in range(3):
            w_ps = ps_misc.tile([128, NSAMP], F32, tag="scr")
            for quad in range(NQUAD):
                for dc in range(2):
                    j = (quad * 3 + m) * 2 + dc
                    perm = cst[:, C_PERM + 128 * j : C_PERM + 128 * j + PCOUNT[m]]
                    nc.tensor.matmul(
                        w_ps[0 : PCOUNT[m], 16 * quad : 16 * quad + 16],
                        perm,
                        ind_sb[:, 16 * dc : 16 * dc + 16],
                        start=(dc == 0),
                        stop=(dc == 1),
                    )
            w = persist.tile([PCOUNT[m], 64], xdt, tag=f"wst{m}")
            nc.scalar.copy(w[:], w_ps[0 : PCOUNT[m], 0:64])
            wst.append(w)

        # ---- main matmul: out[64, 12288] = Wstack.T @ Xtiles ----
        for g0, g1 in FC_GROUPS:
            ptiles = []
            for fc in range(g0, g1):
                pt = ps_mm.tile([64, FCW], F32, tag="mm")
                ptiles.append((fc, pt))
            for cc in range(3):
                for fc, pt in ptiles:
                    nc.tensor.matmul(
                        pt[:],
                        wst[cc][:],
                        xch[cc][:, fc * FCW : (fc + 1) * FCW],
                        start=(cc == 0),
                        stop=(cc == 2),
                    )
            st = stage.tile([64, FCW * (g1 - g0)], F32, tag="st")
            for fc, pt in ptiles:
                nc.scalar.copy(st[:, (fc - g0) * FCW : (fc - g0 + 1) * FCW], pt[:])
            nc.sync.dma_start(out[:, g0 * FCW : g1 * FCW], st[:])

    nc.compile()
    return nc


_NC = None


def _get_program():
    global _NC
    if _NC is None:
        _NC = build_program()
    return _NC


def _host_consts():
    import jax

    cpu = jax.devices("cpu")[0]
    with jax.default_device(cpu):
        key = jax.random.key(42)
        noise = jax.random.normal(key, (N_CORES, NSAMP, D), dtype=np.float32)
        noise = np.asarray(noise) * np.float32(SIGMA)

    blocks = [
        np.eye(128, dtype=np.float32),
        np.triu(np.ones((128, 128), np.float32), k=1),  # [d', d]: 1 iff d' < d
        np.ones((128, 128), np.float32),
    ]
    for quad in range(NQUAD):
        a, bh = quad >> 1, quad & 1
        for m in range(3):
            for dc in range(2):
                P = np.zeros((128, 128), np.float32)
                for i in range(16):
                    for w in range(16):
                        d = 16 * i + w
                        if d // 128 != dc:
                            continue
                        g = 17 * (i + a) + (w + bh)
                        gl = g - 128 * m
                        if 0 <= gl < PCOUNT[m]:
                            P[d % 128, gl] = 1.0
                blocks.append(P)
    bias = np.zeros((128, 17), np.float32)
    for k in range(16):
        bias[:, k] = -(k + 1.0)
    bias[:, 16] = 1.0
    blocks.append(bias)
    consts = np.concatenate(blocks, axis=1)
    return noise, np.ascontiguousarray(consts)


def _prep_core(x_core, scores_core, noise_core, consts):
    """x_core [3,1024,1024] -> tile-major [289, 12288]; pack noise."""
    xp = np.zeros((3, 1088, 1088), np.float32)
    xp[:, 32:1056, 32:1056] = x_core
    xtm = (
        xp.reshape(3, 17, 64, 17, 64)
        .transpose(1, 3, 0, 2, 4)
        .reshape(G_TILES, 12288)
    )
    nzp = noise_core.reshape(NT, NROW, D).transpose(1, 0, 2).reshape(NROW, NT * D)
    blob = np.zeros((128, BLOB_W), np.float32)
    blob[:, 0:CONST_W] = consts
    ublob = np.zeros((128, NT * D + D), np.float32)
    ublob[0:NROW, 0 : NT * D] = nzp
    ublob[0, NT * D : NT * D + D] = scores_core.reshape(D)
    if USE_BF16:
        import ml_dtypes

        xtm = xtm.astype(ml_dtypes.bfloat16)
    return {"xt": np.ascontiguousarray(xtm), "blob": blob, "ublob": ublob}


def kernel(x_high: np.ndarray, scores_2d: np.ndarray) -> np.ndarray:
    nc = _get_program()
    noise, consts = _host_consts()
    x_high = np.asarray(x_high, dtype=np.float32)
    scores_2d = np.asarray(scores_2d, dtype=np.float32)
    in_maps = [
        _prep_core(x_high[c], scores_2d[c], noise[c], consts)
        for c in range(N_CORES)
    ]
    res = run_bass_kernel_spmd(nc, in_maps, list(range(N_CORES)))
    outs = []
    for c in range(N_CORES):
        o = res.results[c]["out"]  # [64, 12288] rows = 16*quad + k
        o = o.reshape(2, 2, 16, 3, 64, 64).transpose(2, 3, 0, 4, 1, 5)
        outs.append(o.reshape(16, 3, 128, 128))
    return np.stack(outs).reshape(128, 3, 128, 128)
